# revision 18
# baseline (speedup 1.0000x reference)
"""Trainium2 Bass kernel for causal multi-head attention with partial RoPE.

Problem (nn_Attn): x[128,512,384] -> qkv proj (Wqkv [1152,384]) -> split
q,k,v into 6 heads of 64 -> partial rotary (first 16 channels) on q,k ->
causal softmax attention -> out proj (Wout [384,384]).

Strategy: data-parallel over batch B across 8 NeuronCores (16 batches per
core). Per batch, on each core:
  A. load x [512,384], PE-transpose to xT [384,512] (f32r data, bf16
     identity moving operand), evac to bf16.
  B. qkT = Wqk @ xT (d-major layout: 6 tiles of [128,512], 2 heads per
     tile). RoPE via DVE: evac copy -> bf16, stream_shuffle partner rows,
     cos/sin mask multiplies (bf16 2x mode), final add on GpSimd.
     v = xT.T @ WvT in [t, d] layout.
  C. attention per head-pair p (2 heads stacked in one 128-tile):
     S^T chunks [k=128, q<=512] via row-packed K=64 matmul pairs
     (tile_position concurrency); exp on ACT (bf16 out); causal diagonal
     zeroed POST-exp via GpSimd affine_select (multiplicative mask);
     O^T[hd, q] via col-packed K=128 matmul pairs; softmax denominators
     via two concurrent M=1 ones-column matmuls (col groups 0 and 32);
     denominator broadcast to 128 partitions with one K=33 f32r matmul;
     normalization via DVE tensor-tensor divide.
  D. y = O^T.T @ WoutT in [t, c] layout, DMA out.

Projections use bf16 weights/activations (1 cycle/row); accumulation is
fp32 in PSUM. Emission is software-pipelined two batches ahead so the
tensor engine never idles long enough to drop out of the HAM K=8/8
(2.4 GHz) clock state.
"""

import os
import sys

for _p in ("/opt/trn_rl_repo",):
    if _p not in sys.path and os.path.isdir(_p):
        sys.path.append(_p)

import numpy as np
import ml_dtypes

import concourse.bass as bass
import concourse.mybir as mybir
import concourse.tile as tile
from concourse.bass import ts
from concourse.bass_utils import run_bass_kernel_spmd
from concourse.vector_clock import ScopedClock

B, T, C, NH, RD = 128, 512, 384, 6, 16
HD = C // NH  # 64
NCORES = 8
BL = B // NCORES  # batches per core
NDT = 2 * C // 128  # 6 qk d-tiles
NTT = T // 128  # 4 t-tiles
NPAIR = NH // 2  # 3 head pairs
F32 = mybir.dt.float32
F32R = mybir.dt.float32r
BF16 = mybir.dt.bfloat16
AF = mybir.ActivationFunctionType
ALU = mybir.AluOpType


import bass_rust as _bass_rust


class TC(tile.TileContext):
    """TileContext adapted to this walrus build, which rejects more than
    one sync-wait command on an instruction: excess waits are split onto
    same-engine NoOps inserted immediately before the instruction."""

    MAX_WAITS = 1

    def _lower_ordered_insts(self, ordered):
        for bb_name, insts in list(ordered.items()):
            out = []
            for inst in insts:
                si = getattr(inst, "sync_info", None)
                waits = list(si.on_wait) if si is not None and si.on_wait else []
                if len(waits) > self.MAX_WAITS:
                    extra = waits[: -self.MAX_WAITS]
                    si.on_wait = waits[-self.MAX_WAITS:]
                    for w in extra:
                        n = _bass_rust.InstNoOp(
                            name=self.nc.get_next_instruction_name(),
                            ins=[], outs=[], engine=inst.engine,
                        )
                        n.sync_info = mybir.SyncInfo(on_wait=[w], on_update=[])
                        out.append(n)
                out.append(inst)
            ordered[bb_name] = out
        super()._lower_ordered_insts(ordered)

    def _drain_and_barrier(self, tick_clock, wait_clock):
        drain_inst = self.nc.sync.drain()
        wait_clock.add_sem_waits(
            drain_inst.ins, ScopedClock({None: tick_clock.global_clock})
        )
        waits = list(drain_inst.ins.sync_info.on_wait)
        if len(waits) > 1:
            drain_inst.ins.sync_info.on_wait = [waits[0]]
            for w in waits[1:]:
                n = self.nc.sync.nop(nofuse=True)
                n.ins.sync_info = mybir.SyncInfo(on_wait=[w], on_update=[])
            self.nc.sync.drain()
        self.nc.all_engine_barrier()
        popped = self.nc._tile_sem_poison_stack.pop()
        assert popped is self._sem_poison
        self.nc.clear_and_free_semaphores(list(self.sems.allocated().values()))
        self.nc.all_engine_barrier()


def build_program(bl=BL):
    nc = bass.Bass("TRN2", target_bir_lowering=False, num_devices=NCORES)

    x_d = nc.dram_tensor("x", [bl, T, C], BF16, kind="ExternalInput")
    wqk_d = nc.dram_tensor("wqkT", [C, 2 * C], BF16, kind="ExternalInput")
    wv_d = nc.dram_tensor("wvT", [C, C], BF16, kind="ExternalInput")
    wo_d = nc.dram_tensor("woutT", [C, C], BF16, kind="ExternalInput")
    cm_d = nc.dram_tensor("cmask", [128, T], BF16, kind="ExternalInput")
    tri_d = nc.dram_tensor("trimask", [128, 256], BF16, kind="ExternalInput")
    sm_d = nc.dram_tensor("smask", [128, T], BF16, kind="ExternalInput")
    idb_d = nc.dram_tensor("identb", [128, 128], BF16, kind="ExternalInput")
    bcw_d = nc.dram_tensor("bcw", [128, 128], F32, kind="ExternalInput")
    y_d = nc.dram_tensor("y", [bl, T, C], F32, kind="ExternalOutput")

    with TC(nc) as tc:
        _body(nc, tc, bl, x_d, wqk_d, wv_d, wo_d, cm_d, sm_d, idb_d,
              bcw_d, y_d, tri_d)
    return nc


def _body(nc, tc, bl, x_d, wqk_d, wv_d, wo_d, cm_d, sm_d, idb_d,
          bcw_d, y_d, tri_d=None):
    import contextlib

    ctx = contextlib.ExitStack()
    singles = ctx.enter_context(tc.tile_pool(name="singles", bufs=1))
    stage = ctx.enter_context(tc.tile_pool(name="stage", bufs=2))
    sb_x = ctx.enter_context(tc.tile_pool(name="sb_x", bufs=16))
    sb_xT = ctx.enter_context(tc.tile_pool(name="sb_xT", bufs=12))
    sb_qk = ctx.enter_context(tc.tile_pool(name="sb_qk", bufs=24))
    sb_rp = ctx.enter_context(tc.tile_pool(name="sb_rp", bufs=12))
    sb_v = ctx.enter_context(tc.tile_pool(name="sb_v", bufs=16))
    sb_p = ctx.enter_context(tc.tile_pool(name="sb_p", bufs=8))
    sb_rd = ctx.enter_context(tc.tile_pool(name="sb_rd", bufs=4))
    sb_ot = ctx.enter_context(tc.tile_pool(name="sb_ot", bufs=8))
    sb_y = ctx.enter_context(tc.tile_pool(name="sb_y", bufs=4))
    # PSUM: 8 banks of 2KB. ps_main [128,512]x2 = 2 banks (transposes, qk
    # projection, v projection, out projection); ps_s [128,1024]x2 = 4
    # banks (S^T pair tiles, denominator broadcast, out projection);
    # ps_o 1 bank; ps_sum 1 bank.
    ps_main = ctx.enter_context(tc.tile_pool(name="ps_main", bufs=2, space="PSUM"))
    ps_s = ctx.enter_context(tc.tile_pool(name="ps_s", bufs=2, space="PSUM"))
    ps_o = ctx.enter_context(tc.tile_pool(name="ps_o", bufs=1, space="PSUM"))
    ps_sum = ctx.enter_context(tc.tile_pool(name="ps_sum", bufs=1, space="PSUM"))

    # ---- constants ----
    def load_const(dram, shape, dtype, tag):
        t = singles.tile(shape, dtype, tag=tag)
        nc.sync.dma_start(out=t, in_=dram.ap())
        return t

    cmask = load_const(cm_d, [128, T], BF16, "cmask")
    smask = load_const(sm_d, [128, T], BF16, "smask")
    identb = load_const(idb_d, [128, 128], BF16, "identb")
    trimask = load_const(tri_d, [128, 2, 128], BF16, "trimask")
    # denominator stationary: M=64 all-ones, so each head's sum matmul
    # writes the denominator broadcast across its 64 output partitions
    # directly (same N-cycle stream cost as M=1)
    ones64 = singles.tile([128, 64], BF16, tag="ones64")
    nc.vector.memset(ones64, 1.0)

    # weights arrive pre-cast to bf16 from the host; plain DMA loads.
    # Emitted after the first batches' x loads/transposes (see below) so
    # the PE has work during the weight DMAs.
    def load_w_bf16(dram, cols, name):
        outs = []
        for cc in range(3):
            wr = singles.tile([128, cols], BF16, tag=f"{name}{cc}")
            nc.sync.dma_start(out=wr, in_=dram[ts(cc, 128), :])
            outs.append(wr)
        return outs

    # RoPE swap permutation: rows p <-> p+8 for p%16 < 8 (within each
    # 16-block); applies the rotary channel pairing to rows 0-15/64-79,
    # with rotation signs folded into smask.
    shuf_mask = list(range(32))
    for i in range(8):
        shuf_mask[i], shuf_mask[i + 8] = shuf_mask[i + 8], shuf_mask[i]

    def emit_load(b):
        """x load, transpose, xT evac (independent of the weights)."""
        xts = []
        for tt in range(NTT):
            xt = sb_x.tile([128, C], BF16, tag="x")
            nc.sync.dma_start(out=xt, in_=x_d[b, ts(tt, 128), :])
            xts.append(xt)
        xT = []
        for cc in range(3):
            tp = ps_main.tile([128, 512], F32, tag="m")
            tpb = tp.bitcast(BF16)  # [128, 1024] bf16 view, same bank
            for tt in range(NTT):
                # 4 bf16 transposes into one PSUM bank: only the first
                # may start=True (start clears the whole 2KB zero-region)
                nc.tensor.matmul(tpb[:, ts(tt, 128)],
                                 xts[tt][:, ts(cc, 128)],
                                 identb, is_transpose=True,
                                 start=(tt == 0), stop=(tt == NTT - 1),
                                 skip_group_check=True)
            xTt = sb_xT.tile([128, T], BF16, tag="xT")
            nc.scalar.activation(xTt, tpb[:, 0:T], AF.Copy)
            xT.append(xTt)
        return xT

    def emit_proj(xT):
        """qk projection + rope, v projection."""
        qkT = []
        for dt in range(NDT):
            qa = ps_main.tile([128, 512], F32, tag="m")
            for cc in range(3):
                nc.tensor.matmul(qa, wqk[cc][:, ts(dt, 128)], xT[cc],
                                 start=(cc == 0), stop=(cc == 2))
            # rope: t = qa*cmask (DVE, doubles as the PSUM evac),
            # shf = partner rows (DVE), m = shf*smask (DVE),
            # qk = t + m (GpSimd, SBUF bf16)
            t_sb = sb_rp.tile([128, T], BF16, tag="ropet")
            shf = sb_rp.tile([128, T], F32, tag="ropes")
            m_sb = sb_rp.tile([128, T], BF16, tag="ropem")
            nc.vector.tensor_mul(t_sb, qa, cmask)
            nc.vector.stream_shuffle(shf, qa, shuf_mask)
            nc.vector.tensor_mul(m_sb, shf, smask)
            qk = sb_qk.tile([128, T], BF16, tag="qk")
            nc.vector.tensor_add(qk, t_sb, m_sb)
            qkT.append(qk)

        vts = []
        for tt in range(NTT):
            vp = ps_main.tile([128, 512], F32, tag="m")
            for cc in range(3):
                nc.tensor.matmul(vp[:, 0:C], xT[cc][:, ts(tt, 128)], wv[cc],
                                 start=(cc == 0), stop=(cc == 2))
            vt = sb_v.tile([128, C], BF16, tag="v")
            nc.vector.tensor_copy(out=vt, in_=vp[:, 0:C])
            vts.append(vt)
        return qkT, vts

    def emit_front(b):
        return emit_proj(emit_load(b))

    def emit_back(b, qkT, vts):
        """Attention per head pair + output projection."""
        oTs = []
        for p in range(NPAIR):
            qt = qkT[p]
            kt = qkT[3 + p]
            ha, hb = 2 * p, 2 * p + 1
            o_ps = ps_o.tile([128, T], F32, tag="o")
            s_ps = ps_sum.tile([128, T], F32, tag="sum")
            # phase 1: all S^T matmuls + exp + causal mask, so the PE
            # never head-of-line blocks on an exp result
            pts = []
            for j in range(NTT):
                qs = 128 * j
                w = T - qs
                st = ps_s.tile([128, 1024], F32, tag="s")
                st_view = st.rearrange("p (h f) -> p h f", h=2)
                # S^T row-packed matmul pair (K=64 each head, array row
                # groups 0/64 -> concurrent)
                nc.tensor.matmul(st[:, 0:w], kt[0:64, ts(j, 128)],
                                 qt[0:64, qs:T], start=True, stop=True,
                                 skip_group_check=True)
                nc.tensor.matmul(st[:, 512:512 + w], kt[64:128, ts(j, 128)],
                                 qt[64:128, qs:T], start=True, stop=True,
                                 skip_group_check=True)
                # exp (both heads in one ACT op), bf16 out
                pt = sb_p.tile([128, 2, 512], BF16, tag="p")
                nc.scalar.activation(pt[:, :, 0:w], st_view[:, :, 0:w], AF.Exp)
                # causal mask on the diagonal 128-block, post-exp
                # multiplicative 0/1 triangle (in-place, DVE bf16)
                nc.vector.tensor_mul(pt[:, :, 0:128], pt[:, :, 0:128],
                                     trimask)
                pts.append(pt)
            # phase 2: O^T accumulation (col-packed) + denominator sums
            # (two concurrent M=1 matmuls at col groups 0 and 32)
            for j in range(NTT):
                qs = 128 * j
                w = T - qs
                pt = pts[j]
                nc.tensor.matmul(o_ps[0:64, qs:T], vts[j][:, ts(ha, HD)],
                                 pt[:, 0, 0:w], start=(j == 0), stop=(j == 3),
                                 skip_group_check=True)
                nc.tensor.matmul(o_ps[64:128, qs:T], vts[j][:, ts(hb, HD)],
                                 pt[:, 1, 0:w], start=(j == 0), stop=(j == 3),
                                 skip_group_check=True)
                nc.tensor.matmul(s_ps[0:64, qs:T], ones64,
                                 pt[:, 0, 0:w], start=(j == 0), stop=(j == 3),
                                 skip_group_check=True)
                nc.tensor.matmul(s_ps[64:128, qs:T], ones64,
                                 pt[:, 1, 0:w], start=(j == 0), stop=(j == 3),
                                 skip_group_check=True)
            # s_ps already holds the denominator broadcast per head;
            # r = exp(-ln d) on ACT (ln and exp share a table set)
            lnd = sb_rd.tile([128, T], F32, tag="lnd")
            nc.scalar.activation(lnd, s_ps, AF.Ln)
            r_sb = sb_rd.tile([128, T], F32, tag="rsb")
            nc.scalar.activation(r_sb, lnd, AF.Exp, scale=-1.0)
            ot = sb_ot.tile([128, T], BF16, tag="ot")
            nc.vector.tensor_mul(ot, o_ps, r_sb)
            oTs.append(ot)

        for tt in range(NTT):
            yp = ps_s.tile([128, 1024], F32, tag="s")
            for p in range(NPAIR):
                nc.tensor.matmul(yp[:, 0:C], oTs[p][:, ts(tt, 128)], wo[p],
                                 start=(p == 0), stop=(p == NPAIR - 1))
            yt = sb_y.tile([128, C], F32, tag="y")
            if tt % 2 == 0:
                nc.vector.tensor_copy(out=yt, in_=yp[:, 0:C])
            else:
                nc.scalar.activation(yt, yp[:, 0:C], AF.Copy)
            nc.sync.dma_start(out=y_d[b, ts(tt, 128), :], in_=yt)

    # software pipeline, 3 batches of lookahead: attention of batch b is
    # emitted BEFORE the projection phase of batch b+3, so the scheduler
    # prefers attention and uses the (already-emitted) front work of
    # later batches as filler during exp/rope waits
    DEPTH = 2
    xTs = [emit_load(i) if i < bl else None for i in range(DEPTH)]
    wqk = load_w_bf16(wqk_d, 2 * C, "wqk")
    wv = load_w_bf16(wv_d, C, "wv")
    wo = load_w_bf16(wo_d, C, "wo")
    states = [emit_proj(xTs[i]) if xTs[i] is not None else None
              for i in range(DEPTH)]
    for b in range(bl):
        emit_back(b, *states[b % DEPTH])
        if b + DEPTH < bl:
            states[b % DEPTH] = emit_front(b + DEPTH)

    ctx.close()


def make_host_consts(Wqkv, Wout, cos, sin):
    Wq = Wqkv[0:C].astype(np.float32) / np.sqrt(np.float32(HD))
    Wk = Wqkv[C:2 * C].astype(np.float32)
    Wv = Wqkv[2 * C:3 * C].astype(np.float32)

    wqkT = np.concatenate([Wq, Wk], axis=0).T.copy()  # [C, 2C]

    wvT = Wv.T.copy()
    woutT = Wout.astype(np.float32).T.copy()

    cosA = np.asarray(cos, np.float32).reshape(T, RD // 2)  # [T, 8]
    sinA = np.asarray(sin, np.float32).reshape(T, RD // 2)
    cmask = np.ones((128, T), np.float32)
    smask = np.zeros((128, T), np.float32)
    for base in (0, 64):
        for i in range(RD):
            cmask[base + i] = cosA[:, i % (RD // 2)]
            # row i (i<8) holds r1_out = r1*cos - r2*sin; the shuffle swaps
            # in r2, so the sin factor is negative there
            sgn = -1.0 if i < RD // 2 else 1.0
            smask[base + i] = sgn * sinA[:, i % (RD // 2)]

    identb = np.eye(128, dtype=ml_dtypes.bfloat16)
    kk, qq = np.meshgrid(np.arange(128), np.arange(128), indexing="ij")
    tri = (qq >= kk).astype(ml_dtypes.bfloat16)  # [k, q] keep lower-left
    trimask = np.concatenate([tri, tri], axis=1).copy()  # both heads
    # broadcast matrix: denominator row 0 -> output partitions 0:64
    # (head a), row 32 -> partitions 64:128 (head b)
    bcw = np.zeros((128, 128), np.float32)
    bcw[0, 0:64] = 1.0
    bcw[32, 64:128] = 1.0

    return dict(wqkT=wqkT.astype(ml_dtypes.bfloat16),
                wvT=wvT.astype(ml_dtypes.bfloat16),
                woutT=woutT.astype(ml_dtypes.bfloat16),
                cmask=cmask.astype(ml_dtypes.bfloat16),
                smask=smask.astype(ml_dtypes.bfloat16),
                identb=identb, trimask=trimask, bcw=bcw)


_CACHE = {}


def prepare(x, Wqkv, Wout, cos, sin):
    if "nc" not in _CACHE:
        _CACHE["nc"] = build_program()
    nc = _CACHE["nc"]
    consts = make_host_consts(np.asarray(Wqkv), np.asarray(Wout), cos, sin)
    x = np.ascontiguousarray(
        np.asarray(x, np.float32).astype(ml_dtypes.bfloat16))
    in_maps = []
    for c in range(NCORES):
        m = dict(consts)
        m["x"] = x[c * BL:(c + 1) * BL]
        in_maps.append(m)
    return nc, in_maps


def run(x, Wqkv, Wout, cos, sin, trace=False):
    nc, in_maps = prepare(x, Wqkv, Wout, cos, sin)
    res = run_bass_kernel_spmd(
        nc, in_maps, core_ids=list(range(NCORES)), trace=trace
    )
    y = np.concatenate([res.results[c]["y"] for c in range(NCORES)], axis=0)
    return y, res


def kernel(x, Wqkv, Wout, cos, sin):
    y, _ = run(x, Wqkv, Wout, cos, sin, trace=False)
    return y


# revision 19
# speedup vs baseline: 1.1432x; 1.1432x over previous
"""Trainium2 Bass kernel for causal multi-head attention with partial RoPE.

Problem (nn_Attn): x[128,512,384] -> qkv proj (Wqkv [1152,384]) -> split
q,k,v into 6 heads of 64 -> partial rotary (first 16 channels) on q,k ->
causal softmax attention -> out proj (Wout [384,384]).

Strategy: data-parallel over batch B across 8 NeuronCores (16 batches per
core). Per batch, on each core:
  A. load x [512,384], PE-transpose to xT [384,512] (f32r data, bf16
     identity moving operand), evac to bf16.
  B. qkT = Wqk @ xT (d-major layout: 6 tiles of [128,512], 2 heads per
     tile). RoPE via DVE: evac copy -> bf16, stream_shuffle partner rows,
     cos/sin mask multiplies (bf16 2x mode), final add on GpSimd.
     v = xT.T @ WvT in [t, d] layout.
  C. attention per head-pair p (2 heads stacked in one 128-tile):
     S^T chunks [k=128, q<=512] via row-packed K=64 matmul pairs
     (tile_position concurrency); exp on ACT (bf16 out); causal diagonal
     zeroed POST-exp via GpSimd affine_select (multiplicative mask);
     O^T[hd, q] via col-packed K=128 matmul pairs; softmax denominators
     via two concurrent M=1 ones-column matmuls (col groups 0 and 32);
     denominator broadcast to 128 partitions with one K=33 f32r matmul;
     normalization via DVE tensor-tensor divide.
  D. y = O^T.T @ WoutT in [t, c] layout, DMA out.

Projections use bf16 weights/activations (1 cycle/row); accumulation is
fp32 in PSUM. Emission is software-pipelined two batches ahead so the
tensor engine never idles long enough to drop out of the HAM K=8/8
(2.4 GHz) clock state.
"""

import os
import sys

for _p in ("/opt/trn_rl_repo",):
    if _p not in sys.path and os.path.isdir(_p):
        sys.path.append(_p)

import numpy as np
import ml_dtypes

import concourse.bass as bass
import concourse.mybir as mybir
import concourse.tile as tile
from concourse.bass import ts
from concourse.bass_utils import run_bass_kernel_spmd
from concourse.vector_clock import ScopedClock

B, T, C, NH, RD = 128, 512, 384, 6, 16
HD = C // NH  # 64
NCORES = 8
BL = B // NCORES  # batches per core
NDT = 2 * C // 128  # 6 qk d-tiles
NTT = T // 128  # 4 t-tiles
NPAIR = NH // 2  # 3 head pairs
F32 = mybir.dt.float32
F32R = mybir.dt.float32r
BF16 = mybir.dt.bfloat16
AF = mybir.ActivationFunctionType
ALU = mybir.AluOpType


import bass_rust as _bass_rust


class TC(tile.TileContext):
    """TileContext adapted to this walrus build, which rejects more than
    one sync-wait command on an instruction: excess waits are split onto
    same-engine NoOps inserted immediately before the instruction."""

    MAX_WAITS = 1

    def _lower_ordered_insts(self, ordered):
        for bb_name, insts in list(ordered.items()):
            out = []
            for inst in insts:
                si = getattr(inst, "sync_info", None)
                waits = list(si.on_wait) if si is not None and si.on_wait else []
                if len(waits) > self.MAX_WAITS:
                    extra = waits[: -self.MAX_WAITS]
                    si.on_wait = waits[-self.MAX_WAITS:]
                    for w in extra:
                        n = _bass_rust.InstNoOp(
                            name=self.nc.get_next_instruction_name(),
                            ins=[], outs=[], engine=inst.engine,
                        )
                        n.sync_info = mybir.SyncInfo(on_wait=[w], on_update=[])
                        out.append(n)
                out.append(inst)
            ordered[bb_name] = out
        super()._lower_ordered_insts(ordered)

    def _drain_and_barrier(self, tick_clock, wait_clock):
        drain_inst = self.nc.sync.drain()
        wait_clock.add_sem_waits(
            drain_inst.ins, ScopedClock({None: tick_clock.global_clock})
        )
        waits = list(drain_inst.ins.sync_info.on_wait)
        if len(waits) > 1:
            drain_inst.ins.sync_info.on_wait = [waits[0]]
            for w in waits[1:]:
                n = self.nc.sync.nop(nofuse=True)
                n.ins.sync_info = mybir.SyncInfo(on_wait=[w], on_update=[])
            self.nc.sync.drain()
        self.nc.all_engine_barrier()
        popped = self.nc._tile_sem_poison_stack.pop()
        assert popped is self._sem_poison
        self.nc.clear_and_free_semaphores(list(self.sems.allocated().values()))
        self.nc.all_engine_barrier()


def build_program(bl=BL):
    nc = bass.Bass("TRN2", target_bir_lowering=False, num_devices=NCORES)

    x_d = nc.dram_tensor("x", [bl, T, C], BF16, kind="ExternalInput")
    wqk_d = nc.dram_tensor("wqkT", [C, 2 * C], BF16, kind="ExternalInput")
    wv_d = nc.dram_tensor("wvT", [C, C], BF16, kind="ExternalInput")
    wo_d = nc.dram_tensor("woutT", [C, C], BF16, kind="ExternalInput")
    cm_d = nc.dram_tensor("cmask", [128, T], BF16, kind="ExternalInput")
    sm_d = nc.dram_tensor("smask", [128, T], BF16, kind="ExternalInput")
    idb_d = nc.dram_tensor("identb", [128, 128], BF16, kind="ExternalInput")
    bcw_d = nc.dram_tensor("bcw", [128, 128], F32, kind="ExternalInput")
    y_d = nc.dram_tensor("y", [bl, T, C], F32, kind="ExternalOutput")

    with TC(nc) as tc:
        _body(nc, tc, bl, x_d, wqk_d, wv_d, wo_d, cm_d, sm_d, idb_d,
              bcw_d, y_d)
    return nc


def _body(nc, tc, bl, x_d, wqk_d, wv_d, wo_d, cm_d, sm_d, idb_d,
          bcw_d, y_d):
    import contextlib

    ctx = contextlib.ExitStack()
    singles = ctx.enter_context(tc.tile_pool(name="singles", bufs=1))
    stage = ctx.enter_context(tc.tile_pool(name="stage", bufs=2))
    sb_x = ctx.enter_context(tc.tile_pool(name="sb_x", bufs=16))
    sb_xT = ctx.enter_context(tc.tile_pool(name="sb_xT", bufs=12))
    sb_qk = ctx.enter_context(tc.tile_pool(name="sb_qk", bufs=24))
    sb_rp = ctx.enter_context(tc.tile_pool(name="sb_rp", bufs=12))
    sb_v = ctx.enter_context(tc.tile_pool(name="sb_v", bufs=16))
    sb_p = ctx.enter_context(tc.tile_pool(name="sb_p", bufs=8))
    sb_rd = ctx.enter_context(tc.tile_pool(name="sb_rd", bufs=4))
    sb_ot = ctx.enter_context(tc.tile_pool(name="sb_ot", bufs=8))
    sb_y = ctx.enter_context(tc.tile_pool(name="sb_y", bufs=4))
    # PSUM: 8 banks of 2KB. ps_main [128,512]x2 = 2 banks (transposes, qk
    # projection, v projection, out projection); ps_s [128,1024]x2 = 4
    # banks (S^T pair tiles, denominator broadcast, out projection);
    # ps_o 1 bank; ps_sum 1 bank.
    ps_main = ctx.enter_context(tc.tile_pool(name="ps_main", bufs=2, space="PSUM"))
    ps_s = ctx.enter_context(tc.tile_pool(name="ps_s", bufs=2, space="PSUM"))
    ps_o = ctx.enter_context(tc.tile_pool(name="ps_o", bufs=1, space="PSUM"))
    ps_sum = ctx.enter_context(tc.tile_pool(name="ps_sum", bufs=1, space="PSUM"))

    # ---- constants ----
    def load_const(dram, shape, dtype, tag):
        t = singles.tile(shape, dtype, tag=tag)
        nc.sync.dma_start(out=t, in_=dram.ap())
        return t

    cmask = load_const(cm_d, [128, T], BF16, "cmask")
    smask = load_const(sm_d, [128, T], BF16, "smask")
    identb = load_const(idb_d, [128, 128], BF16, "identb")
    # denominator stationary: M=64 all-ones, so each head's sum matmul
    # writes the denominator broadcast across its 64 output partitions
    # directly (same N-cycle stream cost as M=1)
    ones64 = singles.tile([128, 64], BF16, tag="ones64")
    nc.vector.memset(ones64, 1.0)

    # weights arrive pre-cast to bf16 from the host; plain DMA loads.
    # Emitted after the first batches' x loads/transposes (see below) so
    # the PE has work during the weight DMAs.
    def load_w_bf16(dram, cols, name):
        outs = []
        for cc in range(3):
            wr = singles.tile([128, cols], BF16, tag=f"{name}{cc}")
            nc.sync.dma_start(out=wr, in_=dram[ts(cc, 128), :])
            outs.append(wr)
        return outs

    # RoPE swap permutation: rows p <-> p+8 for p%16 < 8 (within each
    # 16-block); applies the rotary channel pairing to rows 0-15/64-79,
    # with rotation signs folded into smask.
    shuf_mask = list(range(32))
    for i in range(8):
        shuf_mask[i], shuf_mask[i + 8] = shuf_mask[i + 8], shuf_mask[i]

    def emit_load(b):
        """x load, transpose, xT evac (independent of the weights)."""
        xts = []
        for tt in range(NTT):
            xt = sb_x.tile([128, C], BF16, tag="x")
            nc.sync.dma_start(out=xt, in_=x_d[b, ts(tt, 128), :])
            xts.append(xt)
        xT = []
        for cc in range(3):
            tp = ps_main.tile([128, 512], F32, tag="m")
            tpb = tp.bitcast(BF16)  # [128, 1024] bf16 view, same bank
            for tt in range(NTT):
                # 4 bf16 transposes into one PSUM bank: only the first
                # may start=True (start clears the whole 2KB zero-region)
                nc.tensor.matmul(tpb[:, ts(tt, 128)],
                                 xts[tt][:, ts(cc, 128)],
                                 identb, is_transpose=True,
                                 start=(tt == 0), stop=(tt == NTT - 1),
                                 skip_group_check=True)
            xTt = sb_xT.tile([128, T], BF16, tag="xT")
            nc.scalar.activation(xTt, tpb[:, 0:T], AF.Copy)
            xT.append(xTt)
        return xT

    def emit_proj(xT):
        """qk projection + rope, v projection."""
        qkT = []
        for dt in range(NDT):
            qa = ps_main.tile([128, 512], F32, tag="m")
            for cc in range(3):
                nc.tensor.matmul(qa, wqk[cc][:, ts(dt, 128)], xT[cc],
                                 start=(cc == 0), stop=(cc == 2))
            # rope: t = qa*cmask (DVE, doubles as the PSUM evac),
            # shf = partner rows (DVE), m = shf*smask (DVE),
            # qk = t + m (GpSimd, SBUF bf16)
            t_sb = sb_rp.tile([128, T], BF16, tag="ropet")
            shf = sb_rp.tile([128, T], F32, tag="ropes")
            m_sb = sb_rp.tile([128, T], BF16, tag="ropem")
            nc.vector.tensor_mul(t_sb, qa, cmask)
            nc.vector.stream_shuffle(shf, qa, shuf_mask)
            nc.vector.tensor_mul(m_sb, shf, smask)
            qk = sb_qk.tile([128, T], BF16, tag="qk")
            nc.vector.tensor_add(qk, t_sb, m_sb)
            qkT.append(qk)

        vts = []
        for tt in range(NTT):
            vp = ps_main.tile([128, 512], F32, tag="m")
            for cc in range(3):
                nc.tensor.matmul(vp[:, 0:C], xT[cc][:, ts(tt, 128)], wv[cc],
                                 start=(cc == 0), stop=(cc == 2))
            vt = sb_v.tile([128, C], BF16, tag="v")
            nc.vector.tensor_copy(out=vt, in_=vp[:, 0:C])
            vts.append(vt)
        return qkT, vts

    def emit_front(b):
        return emit_proj(emit_load(b))

    def emit_back(b, qkT, vts):
        """Attention per head pair + output projection."""
        oTs = []
        for p in range(NPAIR):
            qt = qkT[p]
            kt = qkT[3 + p]
            ha, hb = 2 * p, 2 * p + 1
            o_ps = ps_o.tile([128, T], F32, tag="o")
            s_ps = ps_sum.tile([128, T], F32, tag="sum")
            # phase 1: all S^T matmuls + exp + causal mask, so the PE
            # never head-of-line blocks on an exp result
            pts = []
            for j in range(NTT):
                qs = 128 * j
                w = T - qs
                st = ps_s.tile([128, 1024], F32, tag="s")
                st_view = st.rearrange("p (h f) -> p h f", h=2)
                # S^T row-packed matmul pair (K=64 each head, array row
                # groups 0/64 -> concurrent)
                nc.tensor.matmul(st[:, 0:w], kt[0:64, ts(j, 128)],
                                 qt[0:64, qs:T], start=True, stop=True,
                                 skip_group_check=True)
                nc.tensor.matmul(st[:, 512:512 + w], kt[64:128, ts(j, 128)],
                                 qt[64:128, qs:T], start=True, stop=True,
                                 skip_group_check=True)
                # exp (both heads in one ACT op), bf16 out
                pt = sb_p.tile([128, 2, 512], BF16, tag="p")
                nc.scalar.activation(pt[:, :, 0:w], st_view[:, :, 0:w], AF.Exp)
                # causal mask on the diagonal 128-block, post-exp
                # multiplicative: keep where q >= k i.e. f - p >= 0
                nc.gpsimd.affine_select(
                    pt[:, :, 0:128], pt[:, :, 0:128],
                    pattern=[[0, 2], [1, 128]],
                    compare_op=ALU.is_ge, fill=0.0,
                    base=0, channel_multiplier=-1,
                )
                pts.append(pt)
            # phase 2: O^T accumulation (col-packed) + denominator sums
            # (two concurrent M=1 matmuls at col groups 0 and 32)
            for j in range(NTT):
                qs = 128 * j
                w = T - qs
                pt = pts[j]
                nc.tensor.matmul(o_ps[0:64, qs:T], vts[j][:, ts(ha, HD)],
                                 pt[:, 0, 0:w], start=(j == 0), stop=(j == 3),
                                 skip_group_check=True)
                nc.tensor.matmul(o_ps[64:128, qs:T], vts[j][:, ts(hb, HD)],
                                 pt[:, 1, 0:w], start=(j == 0), stop=(j == 3),
                                 skip_group_check=True)
                nc.tensor.matmul(s_ps[0:64, qs:T], ones64,
                                 pt[:, 0, 0:w], start=(j == 0), stop=(j == 3),
                                 skip_group_check=True)
                nc.tensor.matmul(s_ps[64:128, qs:T], ones64,
                                 pt[:, 1, 0:w], start=(j == 0), stop=(j == 3),
                                 skip_group_check=True)
            # s_ps already holds the denominator broadcast per head;
            # r = exp(-ln d) on ACT (ln and exp share a table set)
            lnd = sb_rd.tile([128, T], F32, tag="lnd")
            nc.scalar.activation(lnd, s_ps, AF.Ln)
            r_sb = sb_rd.tile([128, T], F32, tag="rsb")
            nc.scalar.activation(r_sb, lnd, AF.Exp, scale=-1.0)
            ot = sb_ot.tile([128, T], BF16, tag="ot")
            nc.vector.tensor_mul(ot, o_ps, r_sb)
            oTs.append(ot)

        for tt in range(NTT):
            yp = ps_s.tile([128, 1024], F32, tag="s")
            for p in range(NPAIR):
                nc.tensor.matmul(yp[:, 0:C], oTs[p][:, ts(tt, 128)], wo[p],
                                 start=(p == 0), stop=(p == NPAIR - 1))
            yt = sb_y.tile([128, C], F32, tag="y")
            if tt % 2 == 0:
                nc.vector.tensor_copy(out=yt, in_=yp[:, 0:C])
            else:
                nc.scalar.activation(yt, yp[:, 0:C], AF.Copy)
            nc.sync.dma_start(out=y_d[b, ts(tt, 128), :], in_=yt)

    # software pipeline, 3 batches of lookahead: attention of batch b is
    # emitted BEFORE the projection phase of batch b+3, so the scheduler
    # prefers attention and uses the (already-emitted) front work of
    # later batches as filler during exp/rope waits
    DEPTH = 2
    xTs = [emit_load(i) if i < bl else None for i in range(DEPTH)]
    wqk = load_w_bf16(wqk_d, 2 * C, "wqk")
    wv = load_w_bf16(wv_d, C, "wv")
    wo = load_w_bf16(wo_d, C, "wo")
    states = [emit_proj(xTs[i]) if xTs[i] is not None else None
              for i in range(DEPTH)]
    for b in range(bl):
        emit_back(b, *states[b % DEPTH])
        if b + DEPTH < bl:
            states[b % DEPTH] = emit_front(b + DEPTH)

    ctx.close()


def make_host_consts(Wqkv, Wout, cos, sin):
    Wq = Wqkv[0:C].astype(np.float32) / np.sqrt(np.float32(HD))
    Wk = Wqkv[C:2 * C].astype(np.float32)
    Wv = Wqkv[2 * C:3 * C].astype(np.float32)

    wqkT = np.concatenate([Wq, Wk], axis=0).T.copy()  # [C, 2C]

    wvT = Wv.T.copy()
    woutT = Wout.astype(np.float32).T.copy()

    cosA = np.asarray(cos, np.float32).reshape(T, RD // 2)  # [T, 8]
    sinA = np.asarray(sin, np.float32).reshape(T, RD // 2)
    cmask = np.ones((128, T), np.float32)
    smask = np.zeros((128, T), np.float32)
    for base in (0, 64):
        for i in range(RD):
            cmask[base + i] = cosA[:, i % (RD // 2)]
            # row i (i<8) holds r1_out = r1*cos - r2*sin; the shuffle swaps
            # in r2, so the sin factor is negative there
            sgn = -1.0 if i < RD // 2 else 1.0
            smask[base + i] = sgn * sinA[:, i % (RD // 2)]

    identb = np.eye(128, dtype=ml_dtypes.bfloat16)
    # broadcast matrix: denominator row 0 -> output partitions 0:64
    # (head a), row 32 -> partitions 64:128 (head b)
    bcw = np.zeros((128, 128), np.float32)
    bcw[0, 0:64] = 1.0
    bcw[32, 64:128] = 1.0

    return dict(wqkT=wqkT.astype(ml_dtypes.bfloat16),
                wvT=wvT.astype(ml_dtypes.bfloat16),
                woutT=woutT.astype(ml_dtypes.bfloat16),
                cmask=cmask.astype(ml_dtypes.bfloat16),
                smask=smask.astype(ml_dtypes.bfloat16),
                identb=identb, bcw=bcw)


_CACHE = {}


def prepare(x, Wqkv, Wout, cos, sin):
    if "nc" not in _CACHE:
        _CACHE["nc"] = build_program()
    nc = _CACHE["nc"]
    consts = make_host_consts(np.asarray(Wqkv), np.asarray(Wout), cos, sin)
    x = np.ascontiguousarray(
        np.asarray(x, np.float32).astype(ml_dtypes.bfloat16))
    in_maps = []
    for c in range(NCORES):
        m = dict(consts)
        m["x"] = x[c * BL:(c + 1) * BL]
        in_maps.append(m)
    return nc, in_maps


def run(x, Wqkv, Wout, cos, sin, trace=False):
    nc, in_maps = prepare(x, Wqkv, Wout, cos, sin)
    res = run_bass_kernel_spmd(
        nc, in_maps, core_ids=list(range(NCORES)), trace=trace
    )
    y = np.concatenate([res.results[c]["y"] for c in range(NCORES)], axis=0)
    return y, res


def kernel(x, Wqkv, Wout, cos, sin):
    y, _ = run(x, Wqkv, Wout, cos, sin, trace=False)
    return y
